# revision 1
# baseline (speedup 1.0000x reference)
"""MoE layer (top-2 of 8 experts, D=1024, H=4096) on 8 Trainium2 NeuronCores.

Strategy (expert-parallel, per sharding hint):
  - Routing (softmax top-2 over 8 experts) computed on host in float64 from
    the full inputs; tokens are gathered per expert ("all-to-all by routing
    decision" done as part of input sharding).
  - Core e runs expert e's MLP on its routed tokens:
        y = gelu_tanh(x @ W1[e] + b1[e]) @ W2[e]
    as a single Bass/Tile kernel: matmuls in float32r (full PE rate, ~1e-4
    rel err), gelu on the ACT LUT, fp32 PSUM accumulation.
  - Host combines: out[t] = sum_e w[t,e] * (y_e[t] + b2[e]).

Device kernel layout (per core):
  xT [D, CAP] resident in SBUF; W2 [H, D] resident; W1 streamed per
  (chunk, mh). Tokens processed in chunks of 384 so the three mm2 PSUM
  accumulators ([128,1024] = 2 banks each) plus the mm1 accumulator
  ([128,384] = 1 bank, double-buffered) exactly fill the 8 PSUM banks.
  mm1 produces hT [H-tile, tokens] directly (no transposes anywhere), and
  mm2 consumes each hT tile right after its gelu, so hT never needs to be
  fully resident.
"""

import os
import numpy as np

P = 128
CHUNK = 384  # tokens per chunk; 3 mm2 psum tiles * 2 banks + 2 mm1 banks = 8

_BUILD_CACHE = {}
LAST_RESULTS = None  # BassKernelResults of the most recent run (for test.py)


def _routing(x2d, Wg):
    """float64 softmax top-2 routing. Returns (weights [T,E], top2 [T,2])."""
    logits = x2d.astype(np.float64) @ Wg.astype(np.float64)
    m = logits.max(axis=1, keepdims=True)
    p = np.exp(logits - m)
    p /= p.sum(axis=1, keepdims=True)
    top2 = np.argpartition(-p, 2, axis=1)[:, :2]
    w = np.zeros_like(p)
    np.put_along_axis(w, top2, np.take_along_axis(p, top2, axis=1), axis=1)
    return w, top2


GROUP = 4  # H-tiles per group; W1/W2 each stream exactly once


def _build(D, H, cap, repeat=1):
    import concourse.bacc as bacc
    import concourse.mybir as mybir
    import concourse.tile as tile

    key = (D, H, cap, repeat)
    if key in _BUILD_CACHE:
        return _BUILD_CACHE[key]

    kd = D // P            # mm1 contraction tiles (over D)
    mh_tiles = H // P      # H tiles (mm1 output partitions / mm2 contraction)
    ngroups = mh_tiles // GROUP
    nd = D // 512          # mm2 output free-dim slices
    mt_tiles = cap // P    # token tiles
    # token n-slices for mm1 (<=512 each, f32r needs >=256 for full rate)
    ntiles = []
    off = 0
    while off < cap:
        w = min(512, cap - off)
        ntiles.append((off, w))
        off += w

    nc = bacc.Bacc("TRN2", target_bir_lowering=False, debug=False, num_devices=8)
    f32 = mybir.dt.float32
    f32r = mybir.dt.float32r

    d_xT = nc.dram_tensor("xT", [D, cap], f32r, kind="ExternalInput")
    # host-packed W1: w1p[mh, p, ko*128+j] = W1[ko*128+p, mh*128+j]
    d_w1 = nc.dram_tensor("w1p", [mh_tiles, P, D], f32r, kind="ExternalInput")
    d_w2 = nc.dram_tensor("w2", [H, D], f32r, kind="ExternalInput")
    d_b1 = nc.dram_tensor("b1", [H], f32, kind="ExternalInput")
    d_y = nc.dram_tensor("y", [cap, D], f32, kind="ExternalOutput")

    xT_t = d_xT.rearrange("(ko p) n -> p ko n", p=P)
    w2_t = d_w2.rearrange("(ko p) d -> p ko d", p=P)
    b1_t = d_b1.rearrange("(ko p) -> p ko", p=P)

    gelu = mybir.ActivationFunctionType.Gelu_apprx_tanh
    add_op = mybir.AluOpType.add

    with tile.TileContext(nc) as tc:
        with (
            tc.tile_pool(name="resident", bufs=1) as res,
            tc.tile_pool(name="w1s", bufs=2 * GROUP) as w1pool,
            tc.tile_pool(name="w2s", bufs=2 * GROUP) as w2pool,
            tc.tile_pool(name="ht", bufs=2) as htpool,
            tc.tile_pool(name="ph", bufs=3, space="PSUM") as phpool,
            tc.tile_pool(name="py", bufs=3, space="PSUM") as pypool,
        ):
          for rep in range(repeat):
            # resident: xT, b1, y accumulator
            xT_sb = res.tile([P, kd, cap], f32r, tag="xT", name=f"xT_{rep}")
            for k in range(kd):
                nc.sync.dma_start(xT_sb[:, k, :], xT_t[:, k, :])
            b1_sb = res.tile([P, mh_tiles], f32, tag="b1", name=f"b1_{rep}")
            nc.sync.dma_start(b1_sb[:], b1_t[:, :])
            yacc = res.tile([P, mt_tiles, D], f32, tag="yacc", name=f"yacc_{rep}")

            for g in range(ngroups):
                # stream this group's weights (each byte of W1/W2 read once)
                w1g, w2g = [], []
                for mi in range(GROUP):
                    mh = g * GROUP + mi
                    w1t = w1pool.tile([P, kd, P], f32r, tag="w1",
                                      name=f"w1_{rep}_{mh}")
                    nc.sync.dma_start(
                        w1t[:], d_w1[mh].rearrange("p (ko j) -> p ko j", ko=kd))
                    w1g.append(w1t)
                    w2t = w2pool.tile([P, D], f32r, tag="w2",
                                      name=f"w2_{rep}_{mh}")
                    nc.sync.dma_start(w2t[:], w2_t[:, mh, :])
                    w2g.append(w2t)

                # mm1 + gelu: hT for the group's 8 H-tiles, all tokens
                htg = htpool.tile([P, GROUP, cap], f32r, tag="ht",
                                  name=f"ht_{rep}_{g}")
                for mi in range(GROUP):
                    mh = g * GROUP + mi
                    for (n0, nw) in ntiles:
                        ph = phpool.tile([P, 512], f32, tag="ph",
                                         name=f"ph_{rep}_{mh}_{n0}")
                        for k in range(kd):
                            nc.tensor.matmul(
                                ph[:, :nw], w1g[mi][:, k, :],
                                xT_sb[:, k, n0:n0 + nw],
                                start=(k == 0), stop=(k == kd - 1),
                            )
                        nc.scalar.activation(
                            htg[:, mi, n0:n0 + nw], ph[:, :nw], gelu,
                            bias=b1_sb[:, mh:mh + 1],
                        )

                # mm2: accumulate this group's contribution into yacc
                for mt in range(mt_tiles):
                    for n in range(nd):
                        py = pypool.tile([P, 512], f32, tag="py",
                                         name=f"py_{rep}_{g}_{mt}_{n}")
                        for mi in range(GROUP):
                            nc.tensor.matmul(
                                py[:], htg[:, mi, mt * P:(mt + 1) * P],
                                w2g[mi][:, n * 512:(n + 1) * 512],
                                start=(mi == 0), stop=(mi == GROUP - 1),
                            )
                        dst = yacc[:, mt, n * 512:(n + 1) * 512]
                        if g == 0:
                            nc.vector.tensor_copy(dst, py[:])
                        else:
                            nc.vector.tensor_tensor(dst, dst, py[:], add_op)

            for mt in range(mt_tiles):
                nc.sync.dma_start(d_y[mt * P:(mt + 1) * P, :], yacc[:, mt, :])

    nc.finalize()
    _BUILD_CACHE[key] = nc
    return nc


def _pjrt_plumbing(nc):
    """Names/avals/zero-outs for driving nc through _bass_exec_p ourselves.

    We bypass run_bass_kernel_spmd so the inputs can be device_put into
    device HBM first — host-backed buffers are re-streamed over the slow
    host link on every NEFF execution otherwise.
    """
    import jax
    import concourse.bass2jax as b2j
    import concourse.mybir as mybir

    b2j.install_neuronx_cc_hook()
    partition_name = nc.partition_id_tensor.name if nc.partition_id_tensor else None
    in_names, out_names, out_avals, zero_outs = [], [], [], []
    for alloc in nc.m.functions[0].allocations:
        if not isinstance(alloc, mybir.MemoryLocationSet):
            continue
        name = alloc.memorylocations[0].name
        if alloc.kind == "ExternalInput":
            if name != partition_name:
                in_names.append(name)
        elif alloc.kind == "ExternalOutput":
            out_names.append(name)
            shape = tuple(alloc.tensor_shape)
            dtype = mybir.dt.np(alloc.dtype)
            out_avals.append(jax.core.ShapedArray(shape, dtype))
            zero_outs.append(np.zeros(shape, dtype))
    return partition_name, in_names, out_names, out_avals, zero_outs


def _make_fn(nc, n_iters=1):
    """Jitted 8-core shard_map callable running the NEFF n_iters times."""
    import jax
    from jax.sharding import Mesh, PartitionSpec
    from jax.experimental.shard_map import shard_map
    import concourse.bass2jax as b2j

    partition_name, in_names, out_names, out_avals, zero_outs = _pjrt_plumbing(nc)
    all_names = in_names + out_names
    if partition_name is not None:
        all_names = all_names + [partition_name]

    def _body(*args):
        operands = list(args)
        if partition_name is not None:
            operands.append(b2j.partition_id_tensor())
        outs = None
        for _ in range(n_iters):
            outs = b2j._bass_exec_p.bind(
                *operands,
                out_avals=tuple(out_avals),
                in_names=tuple(all_names),
                out_names=tuple(out_names),
                lowering_input_output_aliases=(),
                sim_require_finite=True,
                sim_require_nnan=True,
                nc=nc,
            )
        return tuple(outs)

    devices = jax.devices()[:8]
    mesh = Mesh(np.asarray(devices), ("core",))
    nin = len(in_names) + len(out_names)
    fn = jax.jit(shard_map(
        _body, mesh=mesh,
        in_specs=(PartitionSpec("core"),) * nin,
        out_specs=(PartitionSpec("core"),) * len(out_names),
        check_rep=False,
    ))
    return fn, mesh, in_names, out_names, zero_outs


def _run_spmd(nc, in_maps):
    """Run the finalized nc once on 8 cores with device-resident inputs.

    Returns (list of per-core {out_name: np.ndarray}, device_inputs) so a
    caller can re-run/bench with the same device buffers.
    """
    import jax
    from jax.sharding import NamedSharding, PartitionSpec

    fn, mesh, in_names, out_names, zero_outs = _make_fn(nc, 1)
    concat_in = [
        np.concatenate([np.asarray(in_maps[c][nm]) for c in range(8)], axis=0)
        for nm in in_names
    ] + [np.concatenate([z] * 8, axis=0) for z in zero_outs]
    sharding = NamedSharding(mesh, PartitionSpec("core"))
    dev_in = [jax.device_put(a, sharding) for a in concat_in]
    jax.block_until_ready(dev_in)
    outs = fn(*dev_in)
    jax.block_until_ready(outs)
    results = []
    for c in range(8):
        r = {}
        for i, nm in enumerate(out_names):
            full = np.asarray(outs[i])
            per = full.shape[0] // 8
            r[nm] = full[c * per:(c + 1) * per]
        results.append(r)
    return results, dev_in


def _stage_sets(in_maps, in_names, zero_outs, mesh, nsets):
    """nsets distinct device-resident input tuples (unique buffers per call,
    so no call in a measurement sequence repeats an (exe, buffers) pair)."""
    import jax
    from jax.sharding import NamedSharding, PartitionSpec

    sharding = NamedSharding(mesh, PartitionSpec("core"))
    base = [
        np.concatenate([np.asarray(in_maps[c][nm]) for c in range(8)], axis=0)
        for nm in in_names
    ]
    zo = [np.concatenate([z] * 8, axis=0) for z in zero_outs]
    sets = []
    for s in range(nsets):
        scale = np.float32(1.0 + 0.0625 * s)
        dev = [jax.device_put(a * scale, sharding) for a in base]
        dev += [jax.device_put(z, sharding) for z in zo]
        jax.block_until_ready(dev)
        sets.append(dev)
    return sets


def bench_exec_ns(D, H, cap, in_maps, k2=49, reps=8):
    """Per-execution NEFF time (ns): repeat the kernel body K times inside one
    NEFF; per-exec = (wall_K - wall_1)/(K-1), median over interleaved passes.
    Every call uses a unique input buffer set to defeat result caching."""
    import time as _time
    import jax

    fns = {}
    sets = {}
    for k in (1, k2):
        nc = _build(D, H, cap, repeat=k)
        fn, mesh, in_names, out_names, zero_outs = _make_fn(nc, 1)
        fns[k] = fn
        sets[k] = _stage_sets(in_maps, in_names, zero_outs, mesh, reps + 1)
    # warm both
    for k in (1, k2):
        jax.block_until_ready(fns[k](*sets[k][-1]))
    diffs = []
    w1s, wks = [], []
    for i in range(reps):
        t0 = _time.perf_counter()
        jax.block_until_ready(fns[1](*sets[1][i]))
        w1 = _time.perf_counter() - t0
        t0 = _time.perf_counter()
        jax.block_until_ready(fns[k2](*sets[k2][i]))
        wk = _time.perf_counter() - t0
        w1s.append(w1)
        wks.append(wk)
        diffs.append((wk - w1) / (k2 - 1))
    diffs.sort()
    med = diffs[len(diffs) // 2]
    return med * 1e9


def kernel(x, Wg, W1, b1, W2, b2):
    global LAST_RESULTS

    x = np.asarray(x, dtype=np.float32)
    Wg = np.asarray(Wg, dtype=np.float32)
    W1 = np.asarray(W1, dtype=np.float32)
    b1 = np.asarray(b1, dtype=np.float32)
    W2 = np.asarray(W2, dtype=np.float32)
    b2 = np.asarray(b2, dtype=np.float32)

    B, S, D = x.shape
    E, _, H = W1.shape
    T = B * S
    x2d = np.ascontiguousarray(x.reshape(T, D))

    weights, _top2 = _routing(x2d, Wg)

    idx = [np.nonzero(weights[:, e])[0] for e in range(E)]
    maxn = max(len(i) for i in idx)
    cap = max(P, -(-maxn // P) * P)

    nc = _build(D, H, cap)

    kd, mh_tiles = D // P, H // P
    in_maps = []
    for e in range(E):
        xT = np.zeros((D, cap), dtype=np.float32)
        xT[:, :len(idx[e])] = x2d[idx[e]].T
        # pack W1 so each [P, kd*P] H-block is contiguous per partition:
        # w1p[mh, p, ko*P + j] = W1[e, ko*P + p, mh*P + j]
        w1p = np.ascontiguousarray(
            W1[e].reshape(kd, P, mh_tiles, P)
                 .transpose(2, 1, 0, 3)
                 .reshape(mh_tiles, P, D))
        in_maps.append({
            "xT": xT,
            "w1p": w1p,
            "w2": np.ascontiguousarray(W2[e]),
            "b1": np.ascontiguousarray(b1[e]),
        })

    results, dev_in = _run_spmd(nc, in_maps)
    LAST_RESULTS = {
        "nc": nc, "dev_in": dev_in, "results": results,
        "D": D, "H": H, "cap": cap, "in_maps": in_maps,
    }

    out = weights.astype(np.float32) @ b2  # the b2 term, exact
    out = out.astype(np.float64)
    for e in range(E):
        y_e = results[e]["y"][:len(idx[e])].astype(np.float64)
        out[idx[e]] += weights[idx[e], e][:, None] * y_e
    return out.reshape(B, S, D).astype(np.float32)



# revision 2
# speedup vs baseline: 1.2752x; 1.2752x over previous
"""MoE layer (top-2 of 8 experts, D=1024, H=4096) on 8 Trainium2 NeuronCores.

Strategy (hidden-dim parallel):
  - Routing (softmax top-2 over 8 experts) computed on host in float64.
    The 8192 routed (token, expert) pairs are gathered in expert-major
    order into one column set shared by ALL cores.
  - Core c owns a 512-wide slice of the FFN hidden dim of EVERY expert:
        hT = gelu(W1[e][:, c*512:(c+1)*512].T @ x + b1-slice)
        yT_partial = W2[e][c*512:(c+1)*512, :].T @ hT
    Every core processes all 8192 pairs -> perfect load balance (the
    expert-parallel alternative pads every core to max_e n_e, ~12% waste).
    Tokens are the matmul FREE dim, so per-expert column counts need no
    128-alignment: zero padded compute.
  - Host combines: y = sum_c yT_partial_c, out[t] = sum w[t,e]*(y + b2[e]).

Device kernel (per core, all bf16 operands, fp32 PSUM):
  - W1/W2 slices of all 8 experts resident in SBUF (bf16 makes them fit:
    128 KB/partition), b1 slice resident.
  - Tokens processed in per-expert chunks of <=512 columns (balanced
    widths >=343). Per chunk: mm1 (4 h-tiles x 8 k-tiles), gelu on ACT
    engine (PSUM -> bf16 SBUF), mm2 (8 d-tiles x 4 k-tiles), DVE copy
    PSUM -> bf16 staging, DMA out.
  - Software pipelined: mm2(chunk c) is emitted after mm1(chunk c+1) so
    the tensor engine never waits on gelu. PSUM: 4 mm1 banks + 3 mm2
    banks = 7 of 8. Input DMA on sync (SP ring), output on scalar (ACT
    ring) so store-waits never block input prefetch.
"""

import numpy as np
import ml_dtypes

P = 128
BF16 = ml_dtypes.bfloat16

_BUILD_CACHE = {}
LAST_RESULTS = None  # set by kernel(); test.py uses this for the bench


def _routing(x2d, Wg):
    """float64 softmax top-2 routing. Returns (weights [T,E], top2 [T,2])."""
    logits = x2d.astype(np.float64) @ Wg.astype(np.float64)
    m = logits.max(axis=1, keepdims=True)
    p = np.exp(logits - m)
    p /= p.sum(axis=1, keepdims=True)
    top2 = np.argpartition(-p, 2, axis=1)[:, :2]
    w = np.zeros_like(p)
    np.put_along_axis(w, top2, np.take_along_axis(p, top2, axis=1), axis=1)
    return w, top2


def _chunk_plan(counts):
    """Split each expert's token count into balanced chunks of <=512 cols.

    Returns a tuple of (expert, width) in expert-major order. Column
    offsets are implicit (cumulative); every chunk occupies one 512-col
    padded block in the packed x / y DRAM layout.
    """
    chunks = []
    for e, n in enumerate(counts):
        if n == 0:
            continue
        m = -(-n // 512)
        q, r = divmod(n, m)
        widths = [q + 1] * r + [q] * (m - r)
        for w in widths:
            chunks.append((e, w))
    return tuple(chunks)


def _build(D, H, chunks, repeat=1):
    import concourse.bacc as bacc
    import concourse.mybir as mybir
    import concourse.tile as tile

    key = (D, H, chunks, repeat)
    if key in _BUILD_CACHE:
        return _BUILD_CACHE[key]

    E = 8
    kd = D // P          # 8  mm1 contraction tiles (over D)
    jh = 4               # h-tiles per core (H/8/128)
    nd = D // P          # 8  mm2 output d-tiles
    C = len(chunks)

    nc = bacc.Bacc("TRN2", target_bir_lowering=False, debug=False, num_devices=8)
    f32 = mybir.dt.float32
    bf16 = mybir.dt.bfloat16
    gelu = mybir.ActivationFunctionType.Gelu_apprx_tanh

    d_x = nc.dram_tensor("xp", [P, C, kd, 512], bf16, kind="ExternalInput")
    d_w1 = nc.dram_tensor("w1p", [E, P, kd, 512], bf16, kind="ExternalInput")
    d_w2 = nc.dram_tensor("w2p", [E, P, jh, D], bf16, kind="ExternalInput")
    d_b1 = nc.dram_tensor("b1p", [P, E * jh], f32, kind="ExternalInput")
    d_y = nc.dram_tensor("yp", [P, C, nd, 512], bf16, kind="ExternalOutput")

    with tile.TileContext(nc) as tc:
        with (
            tc.tile_pool(name="wres", bufs=1) as wres,
            tc.tile_pool(name="xs", bufs=3) as xpool,
            tc.tile_pool(name="hs", bufs=2) as hpool,
            tc.tile_pool(name="ys", bufs=2) as ypool,
            tc.tile_pool(name="ph", bufs=4, space="PSUM") as phpool,
            tc.tile_pool(name="py", bufs=3, space="PSUM") as pypool,
        ):
          for rep in range(repeat):
            w1s, w2s = [], []
            for e in range(E):
                t1 = wres.tile([P, kd, 512], bf16, tag=f"w1_{e}",
                               name=f"w1_{e}_{rep}")
                nc.sync.dma_start(t1[:], d_w1[e])
                w1s.append(t1)
                t2 = wres.tile([P, jh, D], bf16, tag=f"w2_{e}",
                               name=f"w2_{e}_{rep}")
                nc.sync.dma_start(t2[:], d_w2[e])
                w2s.append(t2)
            b1s = wres.tile([P, E * jh], f32, tag="b1", name=f"b1_{rep}")
            nc.sync.dma_start(b1s[:], d_b1[:])

            def emit_mm2(e, ci, w, ht):
                st = ypool.tile([P, nd, 512], bf16, tag="st",
                                name=f"st_{rep}_{ci}")
                for n in range(nd):
                    py = pypool.tile([P, 512], f32, tag="py",
                                     name=f"py_{rep}_{ci}_{n}")
                    for kh in range(jh):
                        nc.tensor.matmul(
                            py[:, :w], w2s[e][:, kh, n * P:(n + 1) * P],
                            ht[:, kh, :w],
                            start=(kh == 0), stop=(kh == jh - 1),
                        )
                    nc.vector.tensor_copy(st[:, n, :w], py[:, :w])
                nc.scalar.dma_start(d_y[:, ci], st[:])

            prev = None
            for ci, (e, w) in enumerate(chunks):
                xc = xpool.tile([P, kd, 512], bf16, tag="x",
                                name=f"x_{rep}_{ci}")
                nc.sync.dma_start(xc[:], d_x[:, ci])
                ht = hpool.tile([P, jh, 512], bf16, tag="ht",
                                name=f"ht_{rep}_{ci}")
                for j in range(jh):
                    ph = phpool.tile([P, 512], f32, tag="ph",
                                     name=f"ph_{rep}_{ci}_{j}")
                    for ko in range(kd):
                        nc.tensor.matmul(
                            ph[:, :w], w1s[e][:, ko, j * P:(j + 1) * P],
                            xc[:, ko, :w],
                            start=(ko == 0), stop=(ko == kd - 1),
                        )
                    nc.scalar.activation(
                        ht[:, j, :w], ph[:, :w], gelu,
                        bias=b1s[:, (e * jh + j):(e * jh + j + 1)],
                    )
                if prev is not None:
                    emit_mm2(*prev)
                prev = (e, ci, w, ht)
            emit_mm2(*prev)

    nc.finalize()
    _BUILD_CACHE[key] = nc
    return nc


def _pjrt_plumbing(nc):
    """Names/avals/zero-outs for driving nc through _bass_exec_p ourselves.

    We bypass run_bass_kernel_spmd so the inputs can be device_put into
    device HBM first — host-backed buffers are re-streamed over the slow
    host link on every NEFF execution otherwise.
    """
    import jax
    import concourse.bass2jax as b2j
    import concourse.mybir as mybir

    b2j.install_neuronx_cc_hook()
    partition_name = nc.partition_id_tensor.name if nc.partition_id_tensor else None
    in_names, out_names, out_avals, zero_outs = [], [], [], []
    for alloc in nc.m.functions[0].allocations:
        if not isinstance(alloc, mybir.MemoryLocationSet):
            continue
        name = alloc.memorylocations[0].name
        if alloc.kind == "ExternalInput":
            if name != partition_name:
                in_names.append(name)
        elif alloc.kind == "ExternalOutput":
            out_names.append(name)
            shape = tuple(alloc.tensor_shape)
            dtype = mybir.dt.np(alloc.dtype)
            out_avals.append(jax.core.ShapedArray(shape, dtype))
            zero_outs.append(np.zeros(shape, dtype))
    return partition_name, in_names, out_names, out_avals, zero_outs


def _make_fn(nc, n_iters=1):
    """Jitted 8-core shard_map callable running the NEFF n_iters times."""
    import jax
    from jax.sharding import Mesh, PartitionSpec
    from jax.experimental.shard_map import shard_map
    import concourse.bass2jax as b2j

    partition_name, in_names, out_names, out_avals, zero_outs = _pjrt_plumbing(nc)
    all_names = in_names + out_names
    if partition_name is not None:
        all_names = all_names + [partition_name]

    def _body(*args):
        operands = list(args)
        if partition_name is not None:
            operands.append(b2j.partition_id_tensor())
        outs = None
        for _ in range(n_iters):
            outs = b2j._bass_exec_p.bind(
                *operands,
                out_avals=tuple(out_avals),
                in_names=tuple(all_names),
                out_names=tuple(out_names),
                lowering_input_output_aliases=(),
                sim_require_finite=True,
                sim_require_nnan=True,
                nc=nc,
            )
        return tuple(outs)

    devices = jax.devices()[:8]
    mesh = Mesh(np.asarray(devices), ("core",))
    nin = len(in_names) + len(out_names)
    fn = jax.jit(shard_map(
        _body, mesh=mesh,
        in_specs=(PartitionSpec("core"),) * nin,
        out_specs=(PartitionSpec("core"),) * len(out_names),
        check_rep=False,
    ))
    return fn, mesh, in_names, out_names, zero_outs


def _run_spmd(nc, in_maps):
    """Run the finalized nc once on 8 cores with device-resident inputs."""
    import jax
    from jax.sharding import NamedSharding, PartitionSpec

    fn, mesh, in_names, out_names, zero_outs = _make_fn(nc, 1)
    concat_in = [
        np.concatenate([np.asarray(in_maps[c][nm]) for c in range(8)], axis=0)
        for nm in in_names
    ] + [np.concatenate([z] * 8, axis=0) for z in zero_outs]
    sharding = NamedSharding(mesh, PartitionSpec("core"))
    dev_in = [jax.device_put(a, sharding) for a in concat_in]
    jax.block_until_ready(dev_in)
    outs = fn(*dev_in)
    jax.block_until_ready(outs)
    results = []
    for c in range(8):
        r = {}
        for i, nm in enumerate(out_names):
            full = np.asarray(outs[i])
            per = full.shape[0] // 8
            r[nm] = full[c * per:(c + 1) * per]
        results.append(r)
    return results, dev_in


def _stage_sets(in_maps, in_names, zero_outs, mesh, nsets):
    """nsets distinct device-resident input tuples (unique buffers per call,
    so no call in a measurement sequence repeats an (exe, buffers) pair)."""
    import jax
    from jax.sharding import NamedSharding, PartitionSpec

    sharding = NamedSharding(mesh, PartitionSpec("core"))
    base = [
        np.concatenate([np.asarray(in_maps[c][nm]) for c in range(8)], axis=0)
        for nm in in_names
    ]
    zo = [np.concatenate([z] * 8, axis=0) for z in zero_outs]
    sets = []
    for s in range(nsets):
        scale = np.float32(1.0 + 0.0625 * s)
        dev = [jax.device_put(np.asarray(a * scale, dtype=a.dtype), sharding)
               for a in base]
        dev += [jax.device_put(z, sharding) for z in zo]
        jax.block_until_ready(dev)
        sets.append(dev)
    return sets


def bench_exec_ns(D, H, chunks, in_maps, k2=49, reps=8):
    """Per-execution NEFF time (ns): repeat the kernel body K times inside one
    NEFF; per-exec = (wall_K - wall_1)/(K-1), median over interleaved passes.
    Every call uses a unique input buffer set to defeat result caching."""
    import time as _time
    import jax

    fns = {}
    sets = {}
    for k in (1, k2):
        nc = _build(D, H, chunks, repeat=k)
        fn, mesh, in_names, out_names, zero_outs = _make_fn(nc, 1)
        fns[k] = fn
        sets[k] = _stage_sets(in_maps, in_names, zero_outs, mesh, reps + 1)
    for k in (1, k2):
        jax.block_until_ready(fns[k](*sets[k][-1]))
    diffs = []
    for i in range(reps):
        t0 = _time.perf_counter()
        jax.block_until_ready(fns[1](*sets[1][i]))
        w1 = _time.perf_counter() - t0
        t0 = _time.perf_counter()
        jax.block_until_ready(fns[k2](*sets[k2][i]))
        wk = _time.perf_counter() - t0
        diffs.append((wk - w1) / (k2 - 1))
    diffs.sort()
    med = diffs[len(diffs) // 2]
    return med * 1e9


def kernel(x, Wg, W1, b1, W2, b2):
    global LAST_RESULTS

    x = np.asarray(x, dtype=np.float32)
    Wg = np.asarray(Wg, dtype=np.float32)
    W1 = np.asarray(W1, dtype=np.float32)
    b1 = np.asarray(b1, dtype=np.float32)
    W2 = np.asarray(W2, dtype=np.float32)
    b2 = np.asarray(b2, dtype=np.float32)

    B, S, D = x.shape
    E, _, H = W1.shape
    T = B * S
    HS = H // 8          # per-core hidden slice
    jh = HS // P         # h-tiles per core
    kd = D // P
    nd = D // P
    x2d = np.ascontiguousarray(x.reshape(T, D))

    weights, _top2 = _routing(x2d, Wg)
    idx = [np.nonzero(weights[:, e])[0] for e in range(E)]
    counts = [len(i) for i in idx]
    chunks = _chunk_plan(counts)
    C = len(chunks)

    nc = _build(D, H, chunks)

    # ---- pack inputs ----
    order = np.concatenate(idx)
    xT = np.ascontiguousarray(x2d[order].T).astype(BF16)   # [D, N]
    xp = np.zeros((P, C, kd, 512), dtype=BF16)
    off = 0
    for ci, (e, w) in enumerate(chunks):
        blk = xT[:, off:off + w].reshape(kd, P, w)
        xp[:, ci, :, :w] = blk.transpose(1, 0, 2)
        off += w

    in_maps = []
    for c in range(8):
        w1p = np.empty((E, P, kd, 512), dtype=BF16)
        w2p = np.empty((E, P, jh, D), dtype=BF16)
        b1p = np.empty((P, E * jh), dtype=np.float32)
        for e in range(E):
            sl1 = W1[e][:, c * HS:(c + 1) * HS]            # [D, HS]
            w1p[e] = sl1.reshape(kd, P, HS).transpose(1, 0, 2).astype(BF16)
            sl2 = W2[e][c * HS:(c + 1) * HS, :]            # [HS, D]
            w2p[e] = sl2.reshape(jh, P, D).transpose(1, 0, 2).astype(BF16)
            b1p[:, e * jh:(e + 1) * jh] = (
                b1[e][c * HS:(c + 1) * HS].reshape(jh, P).T)
        in_maps.append({"xp": xp, "w1p": w1p, "w2p": w2p, "b1p": b1p})

    results, dev_in = _run_spmd(nc, in_maps)
    LAST_RESULTS = {
        "nc": nc, "dev_in": dev_in, "results": results,
        "D": D, "H": H, "chunks": chunks, "in_maps": in_maps,
    }

    # ---- combine: sum partial yT over cores, then weighted scatter ----
    ysum = np.zeros((P, C, nd, 512), dtype=np.float32)
    for c in range(8):
        ysum += results[c]["yp"].astype(np.float32)
    yT = np.empty((D, T * 2), dtype=np.float32)            # [D, N]
    off = 0
    for ci, (e, w) in enumerate(chunks):
        blk = ysum[:, ci, :, :w]                           # [P, nd, w]
        yT[:, off:off + w] = blk.transpose(1, 0, 2).reshape(D, w)
        off += w

    out = (weights.astype(np.float64) @ b2.astype(np.float64))
    yfull = yT.T.astype(np.float64)                        # [N, D]
    off = 0
    for e in range(E):
        n = counts[e]
        out[idx[e]] += weights[idx[e], e][:, None] * yfull[off:off + n]
        off += n
    return out.reshape(B, S, D).astype(np.float32)
